# revision 12
# baseline (speedup 1.0000x reference)
"""L1-distance kernel (LPNorm p=1) for Trainium2, 8 NeuronCores.

out[n, hw, o] = sum_c |x[n, hw, c] - w[c, o]| + b[o]
x: (8, 56, 56, 64) f32, w: (64, 128) f32, b: (128,) f32 -> out: (8, 3136, 128) f32

Sharding: data-parallel over batch N; core n handles image n (3136 rows).

Per-core layout: partitions = (c, s), c = 0..63 stacked twice (s = 0/1 handles
output channels 2j / 2j+1), free axis = rows (3136).  Two elementwise
producers run in parallel:
  - ScalarE: |x - w| = Abs(x + bias), per-partition bias -w[c, 2j+s]
  - VectorE: max(x, w) and min(x, w) via single-op tensor_scalar (fp32 2x
    perf mode); sum|x-w| = sum max - sum min via +/-1 selector columns.
TensorE reduces over partitions (contraction = c-stack) with 0/1 (or -1)
selector matmuls accumulating into PSUM so PSUM partition = o.  PSUM is
evacuated to SBUF, DMA'd out as (o, hw); host transposes and adds b.

Built on bacc.Bacc: its event-semaphore pass lowers multi-sem waits (the
plain ISA slot fits one wait per instruction).
"""

import numpy as np

N, H, W, C, OUTC = 8, 56, 56, 64, 128
HW = H * W  # 3136
NCORES = 8
PAIRS = OUTC // 2  # 64
CHUNK = 448  # 3136 = 7 * 448, fits a 2KB fp32 PSUM bank
NCHUNK = HW // CHUNK  # 7

W_OFF = 0  # inp columns [0, 64): +w stacked pairs (VectorE max/min scalars)
NW_OFF = 64  # inp columns [64, 128): -w stacked pairs (ScalarE Abs bias)
SEL_OFF = 128  # inp columns [128, 640): selector source (+1 block, -1 block)
XT_OFF = 640  # x transposed, duplicated
INP_COLS = XT_OFF + HW

N_ACT = 60  # pairs produced by ScalarE; rest by VectorE
AD_DTYPE = "float16"

_CACHE = {}


def _build_bass(n_act=N_ACT, ad_dtype=AD_DTYPE):
    from contextlib import ExitStack

    import concourse.bacc as bacc
    import concourse.mybir as mybir
    from concourse.tile import TileContext

    f32 = mybir.dt.float32
    adt = getattr(mybir.dt, ad_dtype)
    nc = bacc.Bacc("TRN2", target_bir_lowering=False)

    inp = nc.dram_tensor("inp", [128, INP_COLS], f32, kind="ExternalInput")
    out_t = nc.dram_tensor("out_t", [128, HW], f32, kind="ExternalOutput")

    with TileContext(nc) as tc, ExitStack() as ctx:
        consts = ctx.enter_context(tc.tile_pool(name="consts", bufs=1))
        prod_pool = ctx.enter_context(tc.tile_pool(name="prod", bufs=3))
        psum_pool = ctx.enter_context(tc.tile_pool(name="psum", bufs=1, space="PSUM"))

        inp_sb = consts.tile([128, INP_COLS], f32)
        nc.sync.dma_start(out=inp_sb, in_=inp[:, :])
        xt_sb = inp_sb[:, XT_OFF : XT_OFF + HW]

        sel_sb = consts.tile([128, 512], adt)
        nc.vector.tensor_copy(sel_sb, inp_sb[:, SEL_OFF : SEL_OFF + 512])

        out_sb = consts.tile([128, HW], f32)

        ps = [
            psum_pool.tile([128, CHUNK], f32, name=f"ps{k}", tag=f"ps{k}")
            for k in range(NCHUNK)
        ]

        started = [False] * NCHUNK

        def reduce_tiles(j, tiles_and_windows, last_pair):
            for k in range(NCHUNK):
                for ti, (t, (lo, hi)) in enumerate(tiles_and_windows):
                    nc.tensor.matmul(
                        ps[k][:, :],
                        sel_sb[:, lo - 2 * j : hi - 2 * j],
                        t[:, k * CHUNK : (k + 1) * CHUNK],
                        start=not started[k],
                        stop=last_pair and ti == len(tiles_and_windows) - 1,
                    )
                    started[k] = True

        for j in range(PAIRS):
            last = j == PAIRS - 1
            if j < n_act:
                ad = prod_pool.tile([128, HW], adt, name="ad", tag="ad")
                nc.scalar.activation(
                    out=ad,
                    in_=xt_sb,
                    func=mybir.ActivationFunctionType.Abs,
                    bias=inp_sb[:, NW_OFF + j : NW_OFF + j + 1],
                    scale=1.0,
                )
                reduce_tiles(j, [(ad, (128, 256))], last)
            else:
                wj = inp_sb[:, W_OFF + j : W_OFF + j + 1]
                t1 = prod_pool.tile([128, HW], adt, name="t1", tag="t1")
                nc.vector.tensor_scalar(
                    t1, xt_sb, wj, None, mybir.AluOpType.max
                )
                t2 = prod_pool.tile([128, HW], adt, name="t2", tag="t2")
                nc.vector.tensor_scalar(
                    t2, xt_sb, wj, None, mybir.AluOpType.min
                )
                reduce_tiles(j, [(t1, (128, 256)), (t2, (384, 512))], last)

        for k in range(NCHUNK):
            nc.vector.tensor_copy(
                out_sb[:, k * CHUNK : (k + 1) * CHUNK], ps[k][:, :]
            )
        nc.sync.dma_start(out=out_t[:, :], in_=out_sb)

    nc.compile()
    return nc


def _get_nc():
    if "nc" not in _CACHE:
        _CACHE["nc"] = _build_bass()
    return _CACHE["nc"]


def _make_in_maps(x, w):
    base = np.zeros((128, INP_COLS - HW), dtype=np.float32)
    base[:64, W_OFF : W_OFF + PAIRS] = w[:, 0::2]
    base[64:, W_OFF : W_OFF + PAIRS] = w[:, 1::2]
    base[:64, NW_OFF : NW_OFF + PAIRS] = -w[:, 0::2]
    base[64:, NW_OFF : NW_OFF + PAIRS] = -w[:, 1::2]
    # +1 selector block: lhsT window [128-2j, 256-2j)
    base[:64, SEL_OFF + 128] = 1.0
    base[64:, SEL_OFF + 129] = 1.0
    # -1 selector block: lhsT window [384-2j, 512-2j)
    base[:64, SEL_OFF + 384] = -1.0
    base[64:, SEL_OFF + 385] = -1.0

    in_maps = []
    for n in range(NCORES):
        xt = x[n].reshape(HW, C).T  # (64, HW)
        inp = np.empty((128, INP_COLS), dtype=np.float32)
        inp[:, : INP_COLS - HW] = base
        inp[:64, XT_OFF:] = xt
        inp[64:, XT_OFF:] = xt
        in_maps.append({"inp": inp})
    return in_maps


def _run(x, w, b, **run_kwargs):
    from concourse.bass_utils import run_bass_kernel_spmd

    nc = _get_nc()
    in_maps = _make_in_maps(x, w)
    res = run_bass_kernel_spmd(nc, in_maps, core_ids=list(range(NCORES)), **run_kwargs)
    out = np.empty((N, HW, OUTC), dtype=np.float32)
    bias = b.astype(np.float32)[None, :]
    for n in range(NCORES):
        out[n] = res.results[n]["out_t"].T + bias
    return out, res


def kernel(x, w, b):
    x = np.asarray(x, dtype=np.float32)
    w = np.asarray(w, dtype=np.float32)
    b = np.asarray(b, dtype=np.float32)
    out, _ = _run(x, w, b)
    if not np.isfinite(out).all():
        # Cold-NEFF first executions have been observed to return transient
        # garbage once; a re-run on the warm executable is clean.
        out, _ = _run(x, w, b)
    return out


# revision 15
# speedup vs baseline: 1.0092x; 1.0092x over previous
"""L1-distance kernel (LPNorm p=1) for Trainium2, 8 NeuronCores.

out[n, hw, o] = sum_c |x[n, hw, c] - w[c, o]| + b[o]
x: (8, 56, 56, 64) f32, w: (64, 128) f32, b: (128,) f32 -> out: (8, 3136, 128) f32

Sharding: data-parallel over batch N; core n handles image n (3136 rows).

Per-core layout: partitions = (c, s), c = 0..63 stacked twice (s = 0/1 handles
output channels 2j / 2j+1), free axis = rows (3136).  Two elementwise
producers run in parallel:
  - ScalarE: |x - w| = Abs(x + bias), per-partition bias -w[c, 2j+s]
  - VectorE: max(x, w) and min(x, w) via single-op tensor_scalar (fp32 2x
    perf mode); sum|x-w| = sum max - sum min via +/-1 selector columns.
TensorE reduces over partitions (contraction = c-stack) with 0/1 (or -1)
selector matmuls accumulating into PSUM so PSUM partition = o.  PSUM is
evacuated to SBUF, DMA'd out as (o, hw); host transposes and adds b.

Built on bacc.Bacc: its event-semaphore pass lowers multi-sem waits (the
plain ISA slot fits one wait per instruction).
"""

import numpy as np

N, H, W, C, OUTC = 8, 56, 56, 64, 128
HW = H * W  # 3136
NCORES = 8
PAIRS = OUTC // 2  # 64
CHUNK = 448  # 3136 = 7 * 448, fits a 2KB fp32 PSUM bank
NCHUNK = HW // CHUNK  # 7

W_OFF = 0  # inp columns [0, 64): +w stacked pairs (VectorE max/min scalars)
NW_OFF = 64  # inp columns [64, 128): -w stacked pairs (ScalarE Abs bias)
SEL_OFF = 128  # inp columns [128, 640): selector source (+1 block, -1 block)
XT_OFF = 640  # x transposed, duplicated
INP_COLS = XT_OFF + HW

N_ACT = 50  # pairs produced by ScalarE; rest by VectorE
AD_DTYPE = "float16"

_CACHE = {}


def _build_bass(n_act=N_ACT, ad_dtype=AD_DTYPE):
    from contextlib import ExitStack

    import concourse.bacc as bacc
    import concourse.mybir as mybir
    from concourse.tile import TileContext

    f32 = mybir.dt.float32
    adt = getattr(mybir.dt, ad_dtype)
    nc = bacc.Bacc("TRN2", target_bir_lowering=False)

    inp = nc.dram_tensor("inp", [128, INP_COLS], f32, kind="ExternalInput")
    out_t = nc.dram_tensor("out_t", [128, HW], f32, kind="ExternalOutput")

    with TileContext(nc) as tc, ExitStack() as ctx:
        consts = ctx.enter_context(tc.tile_pool(name="consts", bufs=1))
        prod_pool = ctx.enter_context(tc.tile_pool(name="prod", bufs=3))
        psum_pool = ctx.enter_context(tc.tile_pool(name="psum", bufs=1, space="PSUM"))

        inp_sb = consts.tile([128, INP_COLS], f32)
        nc.sync.dma_start(out=inp_sb, in_=inp[:, :])
        xt_sb = inp_sb[:, XT_OFF : XT_OFF + HW]

        sel_sb = consts.tile([128, 512], adt)
        nc.vector.tensor_copy(sel_sb, inp_sb[:, SEL_OFF : SEL_OFF + 512])

        out_sb = consts.tile([128, HW], f32)

        if n_act < PAIRS:
            # fp16 copies of x and w unlock the DVE 4x perf mode (16-bit,
            # single-src, SBUF) for the max/min producer.
            xt16 = consts.tile([128, HW], adt)
            nc.vector.tensor_copy(xt16, xt_sb)

        ps = [
            psum_pool.tile([128, CHUNK], f32, name=f"ps{k}", tag=f"ps{k}")
            for k in range(NCHUNK)
        ]

        started = [False] * NCHUNK

        def reduce_tiles(j, tiles_and_windows, last_pair):
            for k in range(NCHUNK):
                for ti, (t, (lo, hi)) in enumerate(tiles_and_windows):
                    nc.tensor.matmul(
                        ps[k][:, :],
                        sel_sb[:, lo - 2 * j : hi - 2 * j],
                        t[:, k * CHUNK : (k + 1) * CHUNK],
                        start=not started[k],
                        stop=last_pair and ti == len(tiles_and_windows) - 1,
                    )
                    started[k] = True

        for j in range(PAIRS):
            last = j == PAIRS - 1
            if j < n_act:
                ad = prod_pool.tile([128, HW], adt, name="ad", tag="ad")
                nc.scalar.activation(
                    out=ad,
                    in_=xt_sb,
                    func=mybir.ActivationFunctionType.Abs,
                    bias=inp_sb[:, NW_OFF + j : NW_OFF + j + 1],
                    scale=1.0,
                )
                reduce_tiles(j, [(ad, (128, 256))], last)
            else:
                wj = inp_sb[:, W_OFF + j : W_OFF + j + 1]
                t1 = prod_pool.tile([128, HW], adt, name="t1", tag="t1")
                nc.vector.tensor_scalar(
                    t1, xt16, wj, None, mybir.AluOpType.max
                )
                t2 = prod_pool.tile([128, HW], adt, name="t2", tag="t2")
                nc.vector.tensor_scalar(
                    t2, xt16, wj, None, mybir.AluOpType.min
                )
                reduce_tiles(j, [(t1, (128, 256)), (t2, (384, 512))], last)

        for k in range(NCHUNK):
            nc.vector.tensor_copy(
                out_sb[:, k * CHUNK : (k + 1) * CHUNK], ps[k][:, :]
            )
        nc.sync.dma_start(out=out_t[:, :], in_=out_sb)

    nc.compile()
    return nc


def _get_nc():
    if "nc" not in _CACHE:
        _CACHE["nc"] = _build_bass()
    return _CACHE["nc"]


def _make_in_maps(x, w):
    base = np.zeros((128, INP_COLS - HW), dtype=np.float32)
    base[:64, W_OFF : W_OFF + PAIRS] = w[:, 0::2]
    base[64:, W_OFF : W_OFF + PAIRS] = w[:, 1::2]
    base[:64, NW_OFF : NW_OFF + PAIRS] = -w[:, 0::2]
    base[64:, NW_OFF : NW_OFF + PAIRS] = -w[:, 1::2]
    # +1 selector block: lhsT window [128-2j, 256-2j)
    base[:64, SEL_OFF + 128] = 1.0
    base[64:, SEL_OFF + 129] = 1.0
    # -1 selector block: lhsT window [384-2j, 512-2j)
    base[:64, SEL_OFF + 384] = -1.0
    base[64:, SEL_OFF + 385] = -1.0

    in_maps = []
    for n in range(NCORES):
        xt = x[n].reshape(HW, C).T  # (64, HW)
        inp = np.empty((128, INP_COLS), dtype=np.float32)
        inp[:, : INP_COLS - HW] = base
        inp[:64, XT_OFF:] = xt
        inp[64:, XT_OFF:] = xt
        in_maps.append({"inp": inp})
    return in_maps


def _run(x, w, b, **run_kwargs):
    from concourse.bass_utils import run_bass_kernel_spmd

    nc = _get_nc()
    in_maps = _make_in_maps(x, w)
    res = run_bass_kernel_spmd(nc, in_maps, core_ids=list(range(NCORES)), **run_kwargs)
    out = np.empty((N, HW, OUTC), dtype=np.float32)
    bias = b.astype(np.float32)[None, :]
    for n in range(NCORES):
        out[n] = res.results[n]["out_t"].T + bias
    return out, res


def kernel(x, w, b):
    x = np.asarray(x, dtype=np.float32)
    w = np.asarray(w, dtype=np.float32)
    b = np.asarray(b, dtype=np.float32)
    out, _ = _run(x, w, b)
    if not np.isfinite(out).all():
        # Cold-NEFF first executions have been observed to return transient
        # garbage once; a re-run on the warm executable is clean.
        out, _ = _run(x, w, b)
    return out
